# revision 1
# baseline (speedup 1.0000x reference)
"""ChunkedSparseAttention Trainium2 kernel.

Problem: B=2, S=4096, D=1024, CHUNK=64. Per chunk i:
  local  = softmax(Qi @ Ki^T / 32) @ Vi            (own 64 keys)
  cross  = softmax(Qi @ K[:64i]^T / 32) @ V[:64i]  (prefix keys)
  out_i  = local                   if i == 0
         = 0.9 * local + 0.1 * cross otherwise

Distribution: 8 cores, data-parallel over batch (4 cores/batch), with each
core taking one quad-chunk "group" (256 queries) from each of 4 classes
({0-3},{4-7},{8-11},{12-15}) so the triangular prefix work is balanced.
All cores run ONE SPMD NEFF: per-class kb loops are padded to the class max
and masked via a per-core bias table (exp(s/32 + bias), bias=-1e9 kills
padded key blocks). Per-core data differences are handled by host-side
gathers (queries, boundary keys/values, bias, blend coefficients).

On-chip layout ("S^T layout"): scores are computed keys-on-partitions,
S^T[k,q] = sum_d K^T[d,k] Q^T[d,q], so exp(S^T) is directly the lhsT of the
P@V matmul (no transposes on the critical path) and softmax denominators
come from a ones-column matmul. K^T/Q^T are pre-transposed on the host.
Matmuls run in float32r (full PE rate at N>=256, ~1.6e-4 rel err).
"""
import sys

for _p in ("/opt/trn_rl_repo", "/root/.axon_site/_ro/trn_rl_repo"):
    if _p not in sys.path:
        sys.path.insert(0, _p)

import numpy as np

import concourse.bass as bass
import concourse.mybir as mybir
import concourse.tile as tile
from concourse import bacc
from concourse.bass_utils import run_bass_kernel_spmd

F32 = mybir.dt.float32
F32R = mybir.dt.float32r
AF = mybir.ActivationFunctionType
SCALE = 1.0 / 32.0  # 1/sqrt(D)
NEG = -1e9


class Cfg:
    def __init__(self, S, classes):
        self.S = S
        self.D = 1024
        self.classes = classes            # list of 4 lists of group indices
        self.n_slot = len(classes)
        self.M = [2 * max(c) for c in classes]   # padded full-kb count per slot
        self.M = [max(m, 2) for m in self.M]
        self.maxM = max(self.M)
        self.GQ = 256                      # queries per group (4 chunks)
        self.NQ = self.n_slot * self.GQ    # queries per core
        self.n_dblk = self.D // 128
        self.cores_per_batch = len(classes[0])
        self.n_cores = 2 * self.cores_per_batch


FULL = Cfg(4096, [[0, 1, 2, 3], [4, 5, 6, 7], [8, 9, 10, 11], [12, 13, 14, 15]])
MINI = Cfg(1024, [[0], [1], [2], [3]])


def build_nc(cfg: Cfg):
    S, D = cfg.S, cfg.D
    NDB = cfg.n_dblk
    nc = bacc.Bacc("TRN2", target_bir_lowering=False, debug=False)

    kt_in = nc.dram_tensor("kt_in", [D, S], F32, kind="ExternalInput")
    qt_in = nc.dram_tensor("qt_in", [D, cfg.NQ], F32, kind="ExternalInput")
    kbt_in = nc.dram_tensor("kbt_in", [D, cfg.NQ], F32, kind="ExternalInput")
    v_in = nc.dram_tensor("v_in", [S, D], F32, kind="ExternalInput")
    vb_in = nc.dram_tensor("vb_in", [cfg.NQ, D], F32, kind="ExternalInput")
    bias_in = nc.dram_tensor("bias_in", [cfg.n_slot, 128, cfg.maxM], F32,
                             kind="ExternalInput")
    blend_in = nc.dram_tensor("blend_in", [cfg.n_slot, 128, 4], F32,
                              kind="ExternalInput")
    out_t = nc.dram_tensor("out_core", [cfg.NQ, D], F32, kind="ExternalOutput")
    dbg = getattr(cfg, "debug", False)
    if dbg:
        dbg_s = nc.dram_tensor("dbg_sums", [cfg.n_slot, 2, 128, 4], F32,
                               kind="ExternalOutput")
        dbg_o = nc.dram_tensor("dbg_o", [cfg.n_slot, 2, 128, D], F32,
                               kind="ExternalOutput")
    ones_dr = nc.inline_tensor(np.ones((128, 2), np.float32), "ones_c")

    with tile.TileContext(nc) as tc:
        with (
            tc.tile_pool(name="const", bufs=1) as cpool,
            tc.tile_pool(name="kt", bufs=1) as ktp,
            tc.tile_pool(name="qt", bufs=2) as qtp,
            tc.tile_pool(name="kbt", bufs=1) as kbtp,
            tc.tile_pool(name="vb", bufs=1) as vbp,
            tc.tile_pool(name="vsrc", bufs=3) as vsp,
            tc.tile_pool(name="vcast", bufs=3) as vcp,
            tc.tile_pool(name="et", bufs=4) as etp,
            tc.tile_pool(name="eb", bufs=3) as ebp,
            tc.tile_pool(name="bias", bufs=3) as biasp,
            tc.tile_pool(name="blend", bufs=2) as blp,
            tc.tile_pool(name="vec", bufs=10) as vecp,
            tc.tile_pool(name="outst", bufs=3) as outp,
            tc.tile_pool(name="poc", bufs=2, space="PSUM") as poc,
            tc.tile_pool(name="pst", bufs=2, space="PSUM") as pst,
            tc.tile_pool(name="psm", bufs=2, space="PSUM") as psm,
        ):
            ones_t = cpool.tile([128, 2], F32R)
            nc.gpsimd.dma_start(ones_t[:], ones_dr[:])
            ones_f32 = ones_t[:].bitcast(F32)

            # resident K^T, DMA-cast to f32r: [128(d), NDB, S]. Split along S
            # so early score matmuls only wait on the first column ranges
            # instead of the whole 16MB transfer.
            kt = ktp.tile([128, NDB, S], F32R)
            for i in range(8):
                c0, c1 = i * (S // 8), (i + 1) * (S // 8)
                nc.gpsimd.dma_start(
                    kt[:, :, c0:c1],
                    kt_in[:, c0:c1].rearrange("(db p) s -> p db s", p=128))

            for j in range(cfg.n_slot):
                Mj = cfg.M[j]
                qcol = j * cfg.GQ

                # per-slot Q^T, Kb^T (DMA-cast from host-transposed gathers)
                qt = qtp.tile([128, NDB, cfg.GQ], F32R)
                nc.gpsimd.dma_start(
                    qt[:], qt_in[:, qcol:qcol + cfg.GQ]
                    .rearrange("(db p) q -> p db q", p=128))
                kbt = kbtp.tile([128, NDB, cfg.GQ], F32R)
                nc.gpsimd.dma_start(
                    kbt[:], kbt_in[:, qcol:qcol + cfg.GQ]
                    .rearrange("(db p) q -> p db q", p=128))
                vb = vbp.tile([128, 2, D], F32R)
                nc.gpsimd.dma_start(
                    vb[:], vb_in[qcol:qcol + cfg.GQ, :]
                    .rearrange("(c p) d -> p c d", p=128))
                blend = blp.tile([128, 4], F32)
                nc.sync.dma_start(blend[:], blend_in[j])
                bias_slot = biasp.tile([128, cfg.maxM], F32)
                nc.sync.dma_start(bias_slot[:], bias_in[j])

                oc = [poc.tile([128, D], F32, tag="oc", name=f"oc{s}_{j}")
                      for s in range(2)]
                # one PSUM bank per accumulation chain: a second chain's
                # start=True in the same bank clobbers the first chain's
                # has_written state, so each sub's running sums gets its own
                # bank-padded tile.
                sums_c = [psm.tile([128, 2], F32, tag="sums", name=f"sc{s}_{j}")
                          for s in range(2)]

                # ---- full-kb loop (software-pipelined: QK(kb) then PV(kb-1))
                ets = {}
                vts = {}

                def emit_qk(kb):
                    vsrc = vsp.tile([128, D], F32)
                    nc.sync.dma_start(vsrc[:], v_in[kb * 128:(kb + 1) * 128, :])
                    vt = vcp.tile([128, D], F32R)
                    nc.vector.tensor_copy(vt[:], vsrc[:])
                    vts[kb] = vt
                    st = pst.tile([128, cfg.GQ], F32, tag="st")
                    for db in range(NDB):
                        nc.tensor.matmul(
                            st[:], kt[:, db, kb * 128:(kb + 1) * 128],
                            qt[:, db, :], start=(db == 0), stop=(db == NDB - 1))
                    et = etp.tile([128, cfg.GQ], F32R)
                    nc.scalar.activation(et[:], st[:], AF.Exp,
                                         bias=bias_slot[:, kb:kb + 1],
                                         scale=SCALE)
                    ets[kb] = et

                def emit_pv(kb):
                    et, vt = ets.pop(kb), vts.pop(kb)
                    for sub in range(2):
                        lhs = et[:, sub * 128:(sub + 1) * 128]
                        for dh in range(2):
                            nc.tensor.matmul(
                                oc[sub][:, dh * 512:(dh + 1) * 512], lhs,
                                vt[:, dh * 512:(dh + 1) * 512],
                                start=(kb == 0), stop=False)
                        nc.tensor.matmul(sums_c[sub][:], lhs,
                                         ones_t[:], start=(kb == 0), stop=False)

                for kb in range(Mj + 1):
                    if kb < Mj:
                        emit_qk(kb)
                    if kb >= 1:
                        emit_pv(kb - 1)

                # ---- boundary blocks b0/b1 (the group's own 256 keys)
                ebs = []
                for blk in range(2):
                    st = pst.tile([128, cfg.GQ], F32, tag="st")
                    for db in range(NDB):
                        nc.tensor.matmul(
                            st[:], kbt[:, db, blk * 128:(blk + 1) * 128],
                            qt[:, db, :], start=(db == 0), stop=(db == NDB - 1))
                    eb = ebp.tile([128, cfg.GQ], F32R)
                    nc.scalar.activation(eb[:], st[:], AF.Exp, scale=SCALE)
                    ebs.append(eb)
                eb0, eb1 = ebs

                # cross pieces within the boundary:
                # q1 (chunk 4g+1) <- first half of b0; dst partitions 64:128 -> fp32
                for dh in range(2):
                    nc.tensor.matmul(
                        oc[0][64:128, dh * 512:(dh + 1) * 512],
                        eb0[0:64, 64:128].bitcast(F32),
                        vb[0:64, 0, dh * 512:(dh + 1) * 512].bitcast(F32),
                        start=False, stop=(dh == 1))
                nc.tensor.matmul(sums_c[0][64:128, :],
                                 eb0[0:64, 64:128].bitcast(F32),
                                 ones_f32[0:64, :], start=False, stop=True)
                # q2,q3 <- all of b0; dst partitions 0:128 -> f32r
                for dh in range(2):
                    nc.tensor.matmul(
                        oc[1][:, dh * 512:(dh + 1) * 512],
                        eb0[:, 128:256], vb[:, 0, dh * 512:(dh + 1) * 512],
                        start=False, stop=False)
                nc.tensor.matmul(sums_c[1][:], eb0[:, 128:256], ones_t[:],
                                 start=False, stop=False)
                # q3 <- first half of b1; dst partitions 64:128 -> fp32
                for dh in range(2):
                    nc.tensor.matmul(
                        oc[1][64:128, dh * 512:(dh + 1) * 512],
                        eb1[0:64, 192:256].bitcast(F32),
                        vb[0:64, 1, dh * 512:(dh + 1) * 512].bitcast(F32),
                        start=False, stop=(dh == 1))
                nc.tensor.matmul(sums_c[1][64:128, :],
                                 eb1[0:64, 192:256].bitcast(F32),
                                 ones_f32[0:64, :], start=False, stop=True)

                # ---- flush cross, then local per sub (L reuses oc pool slots)
                sums_l = pst.tile([128, 4], F32, tag="st", name=f"sl_{j}")
                for sub in range(2):
                    eb = ebs[sub]
                    # cross normalization * alpha
                    scm = vecp.tile([128, 1], F32, tag="v")
                    nc.vector.tensor_scalar_max(
                        scm[:], sums_c[sub][:, 0:1], 1e-30)
                    rc = vecp.tile([128, 1], F32, tag="v")
                    nc.vector.reciprocal(rc[:], scm[:])
                    rc2 = vecp.tile([128, 1], F32, tag="v")
                    nc.vector.tensor_mul(rc2[:], rc[:],
                                         blend[:, 2 * sub + 1:2 * sub + 2])
                    cs = outp.tile([128, D], F32, tag="out")
                    nc.scalar.activation(cs[:], oc[sub][:], AF.Copy,
                                         scale=rc2[:])
                    if dbg:
                        dso = outp.tile([128, D], F32, tag="out")
                        nc.vector.tensor_copy(dso[:], oc[sub][:])
                        nc.sync.dma_start(dbg_o[j, sub], dso[:])
                        dss = vecp.tile([128, 2], F32, tag="dv", name="dss")
                        nc.vector.tensor_copy(dss[:], sums_c[sub][:])
                        nc.sync.dma_start(dbg_s[j, 0, :, 2 * sub:2 * sub + 2],
                                          dss[:])

                    # local attention for the two chunks of this sub
                    L = poc.tile([128, D], F32, tag="oc")
                    for dh in range(2):
                        nc.tensor.matmul(  # even chunk: partitions 0:64, f32r
                            L[0:64, dh * 512:(dh + 1) * 512],
                            eb[0:64, sub * 128:sub * 128 + 64],
                            vb[0:64, sub, dh * 512:(dh + 1) * 512],
                            start=True, stop=True)
                        nc.tensor.matmul(  # odd chunk: partitions 64:128, fp32
                            L[64:128, dh * 512:(dh + 1) * 512],
                            eb[64:128, sub * 128 + 64:sub * 128 + 128]
                            .bitcast(F32),
                            vb[64:128, sub, dh * 512:(dh + 1) * 512]
                            .bitcast(F32),
                            start=True, stop=True)
                    nc.tensor.matmul(sums_l[0:64, 2 * sub:2 * sub + 2],
                                     eb[0:64, sub * 128:sub * 128 + 64],
                                     ones_t[0:64, :], start=True, stop=True)
                    nc.tensor.matmul(sums_l[64:128, 2 * sub:2 * sub + 2],
                                     eb[64:128, sub * 128 + 64:sub * 128 + 128]
                                     .bitcast(F32),
                                     ones_f32[64:128, :], start=True, stop=True)

                    if dbg and sub == 1:
                        dsl = vecp.tile([128, 4], F32, tag="dv", name="dsl")
                        nc.vector.tensor_copy(dsl[:], sums_l[:])
                        nc.sync.dma_start(dbg_s[j, 1], dsl[:])
                    slm = vecp.tile([128, 1], F32, tag="v")
                    nc.vector.tensor_scalar_max(
                        slm[:], sums_l[:, 2 * sub:2 * sub + 1], 1e-30)
                    rl = vecp.tile([128, 1], F32, tag="v")
                    nc.vector.reciprocal(rl[:], slm[:])
                    rl2 = vecp.tile([128, 1], F32, tag="v")
                    nc.vector.tensor_mul(rl2[:], rl[:],
                                         blend[:, 2 * sub:2 * sub + 1])
                    lt = outp.tile([128, D], F32, tag="out")
                    nc.vector.tensor_scalar_mul(lt[:], L[:], rl2[:])
                    fin = outp.tile([128, D], F32, tag="out")
                    nc.vector.tensor_add(fin[:], lt[:], cs[:])
                    row = (2 * j + sub) * 128
                    nc.sync.dma_start(out_t[row:row + 128, :], fin[:])
    nc.compile()
    return nc


def _host_inputs(cfg: Cfg, query, key, value):
    """Build the 2*cores_per_batch per-core input maps."""
    in_maps = []
    for core in range(cfg.n_cores):
        b = core // cfg.cores_per_batch
        qt_idx = core % cfg.cores_per_batch
        groups = [cls[qt_idx] for cls in cfg.classes]
        kt_core = np.ascontiguousarray(key[b].T)
        q_rows = np.concatenate(
            [query[b, g * cfg.GQ:(g + 1) * cfg.GQ] for g in groups])
        kb_rows = np.concatenate(
            [key[b, g * cfg.GQ:(g + 1) * cfg.GQ] for g in groups])
        vb_rows = np.concatenate(
            [value[b, g * cfg.GQ:(g + 1) * cfg.GQ] for g in groups])
        bias = np.zeros((cfg.n_slot, 128, cfg.maxM), np.float32)
        blend = np.zeros((cfg.n_slot, 128, 4), np.float32)
        for j, g in enumerate(groups):
            bias[j, :, 2 * g:] = NEG
            for sub in range(2):
                for half in range(2):
                    chunk = 4 * g + 2 * sub + half
                    sl = slice(half * 64, half * 64 + 64)
                    blend[j, sl, 2 * sub] = 1.0 if chunk == 0 else 0.9
                    blend[j, sl, 2 * sub + 1] = 0.0 if chunk == 0 else 0.1
        in_maps.append({
            "kt_in": kt_core,
            "qt_in": np.ascontiguousarray(q_rows.T),
            "kbt_in": np.ascontiguousarray(kb_rows.T),
            "v_in": np.ascontiguousarray(value[b]),
            "vb_in": vb_rows,
            "bias_in": bias,
            "blend_in": blend,
        })
    return in_maps


def _scatter_output(cfg: Cfg, results, B):
    out = np.empty((B, cfg.S, cfg.D), np.float32)
    for core in range(cfg.n_cores):
        b = core // cfg.cores_per_batch
        qt_idx = core % cfg.cores_per_batch
        groups = [cls[qt_idx] for cls in cfg.classes]
        oc = results[core]["out_core"]
        for j, g in enumerate(groups):
            out[b, g * cfg.GQ:(g + 1) * cfg.GQ] = oc[j * cfg.GQ:(j + 1) * cfg.GQ]
    return out


_nc_cache = {}


def run(cfg: Cfg, query, key, value, trace=False, trace_kwargs=None):
    ck = (cfg.S, getattr(cfg, "debug", False))
    if ck not in _nc_cache:
        _nc_cache[ck] = build_nc(cfg)
    nc = _nc_cache[ck]
    in_maps = _host_inputs(cfg, query, key, value)
    kw = {}
    if trace:
        kw = dict(trace=True, trace_cores=list(range(cfg.n_cores)),
                  **(trace_kwargs or {}))
    res = run_bass_kernel_spmd(nc, in_maps, core_ids=list(range(cfg.n_cores)),
                               **kw)
    out = _scatter_output(cfg, res.results, query.shape[0])
    return out, res


def kernel(query, key, value):
    query = np.asarray(query, np.float32)
    key = np.asarray(key, np.float32)
    value = np.asarray(value, np.float32)
    out, _ = run(FULL, query, key, value)
    return out



# revision 7
# speedup vs baseline: 2.0136x; 2.0136x over previous
"""ChunkedSparseAttention Trainium2 kernel (fp8/fp16 rewrite).

Problem: B=2, S=4096, D=1024, CHUNK=64. Per chunk i:
  local  = softmax(Qi @ Ki^T / 32) @ Vi            (own 64 keys)
  cross  = softmax(Qi @ K[:64i]^T / 32) @ V[:64i]  (prefix keys)
  out_i  = local                     if i == 0
         = 0.9 * local + 0.1 * cross otherwise

Distribution (same as the f32r baseline): 8 cores, data-parallel over batch
(4 cores/batch); each core takes one 4-chunk group (256 queries) from each of
4 classes ({0-3},{4-7},{8-11},{12-15}) so triangular prefix work is balanced.
One SPMD NEFF: per-class key-block loops padded to the class max; padded
key-block PAIRS are masked via a per-core bias table (exp(s/32 + b), b=-1e9
kills a padded pair, b=-2 on real pairs keeps exp() within fp8e4 range — the
shift cancels in the softmax ratio).

Precision strategy (correctness gate is rel_err < 2e-2):
  - cross/prefix path (alpha=0.1): fp8e4 everywhere with DoubleRow matmuls
    (2 key-blocks contracted per instruction; QK contracts d in 4 insts/kb,
    PV contracts 256 keys/inst). K^T and V live in SBUF in fp8 (3.75MB each).
  - local path (weight 0.9) + boundary scores: fp16 (1 cycle/row like bf16
    but 8x the mantissa). Local PV uses masked fp16 exp tiles (diag blocks,
    off-diag zeroed) so the matmuls run with the full 128-partition dim.
  - output: fp16 (halves the out DMA; 4.9e-4 rel rounding).
The boundary cross pieces are folded into the main cross accumulators as one
extra DoubleRow pair per slot (masked exp rectangles), removing the baseline's
fp32 bitcast boundary matmuls entirely.

PSUM (8 banks): oc[2 subs] 4 banks + score pipeline 3 banks + sums 1 bank.
The sums bank holds 4 accumulation chains (cross/local x 2 subs) using the
PSUM zero-region semantics: only the very first matmul uses start=True, later
chains start=False and read-as-zero (validated on this stack).
"""
import sys

for _p in ("/opt/trn_rl_repo", "/root/.axon_site/_ro/trn_rl_repo"):
    if _p not in sys.path:
        sys.path.insert(0, _p)

import numpy as np
import ml_dtypes

import concourse.bass as bass
import concourse.mybir as mybir
import concourse.tile as tile
from concourse import bacc
from concourse.bass_utils import run_bass_kernel_spmd

F32 = mybir.dt.float32
F16 = mybir.dt.float16
F8 = mybir.dt.float8e4
AF = mybir.ActivationFunctionType
DR = mybir.MatmulPerfMode.DoubleRow
NPF8 = ml_dtypes.float8_e4m3
SCALE = 1.0 / 32.0  # 1/sqrt(D)
NEG = -1e9
SHIFT = -2.0        # exp shift on the cross path; cancels in softmax


class Cfg:
    def __init__(self, S, classes):
        self.S = S
        self.D = 1024
        self.classes = classes              # 4 lists of group indices
        self.n_slot = len(classes)
        self.P = [max(max(c), 1) for c in classes]  # padded PAIR count / slot
        self.maxP = max(self.P)
        self.npair = self.S // 256          # total key pairs in sequence
        self.GQ = 256                       # queries per group (4 chunks)
        self.NQ = self.n_slot * self.GQ     # queries per core
        self.cores_per_batch = len(classes[0])
        self.n_cores = 2 * self.cores_per_batch


FULL = Cfg(4096, [[0, 1, 2, 3], [4, 5, 6, 7], [8, 9, 10, 11], [12, 13, 14, 15]])
MINI = Cfg(1024, [[0], [1], [2], [3]])


def build_nc(cfg: Cfg):
    D = cfg.D
    NS = cfg.n_slot
    KP = min(cfg.maxP, cfg.npair)           # resident K/V pair count
    nc = bacc.Bacc("TRN2", target_bir_lowering=False, debug=False)

    # ---- external inputs (host pre-laid-out, partition-major) ----
    # kt8[p, t, e, dp, u, kk] = K[(2t+e)*128+kk, dp*256+u*128+p]
    kt8_in = nc.dram_tensor("kt8_in", [128, KP, 2, 4, 2, 128], F8,
                            kind="ExternalInput")
    # v8[p, t, e, d] = V[(2t+e)*128+p, d]
    v8_in = nc.dram_tensor("v8_in", [128, KP, 2, D], F8, kind="ExternalInput")
    # qt16[p, j, dp, u, q] = Q[group_j q, dp*256+u*128+p]
    qt16_in = nc.dram_tensor("qt16_in", [128, NS, 4, 2, 256], F16,
                             kind="ExternalInput")
    # qt8 for slot 0 only (avoids gating PE start on the DVE cast)
    qt8s0_in = nc.dram_tensor("qt8s0_in", [128, 4, 2, 256], F8,
                              kind="ExternalInput")
    # kbt16[p, j, dp, u, blk, kk] = K[group_j key blk*128+kk, dp*256+u*128+p]
    kbt16_in = nc.dram_tensor("kbt16_in", [128, NS, 4, 2, 2, 128], F16,
                              kind="ExternalInput")
    # vb16[p, j, e, d] = V[group_j key e*128+p, d]
    vb16_in = nc.dram_tensor("vb16_in", [128, NS, 2, D], F16,
                             kind="ExternalInput")
    # biasp[p, j, t]: -2 (real pair) / -1e9 (padded pair)
    biasp_in = nc.dram_tensor("biasp_in", [128, NS, cfg.maxP], F32,
                              kind="ExternalInput")
    # blend[p, j, 2s]=local coeff, [p, j, 2s+1]=alpha
    blend_in = nc.dram_tensor("blend_in", [128, NS, 4], F32,
                              kind="ExternalInput")
    ones8_in = nc.dram_tensor("ones8_in", [128, 2, 2], F8, kind="ExternalInput")
    ones16_in = nc.dram_tensor("ones16_in", [128, 2], F16, kind="ExternalInput")
    out_t = nc.dram_tensor("out_core", [cfg.NQ, D], F16, kind="ExternalOutput")

    with tile.TileContext(nc) as tc:
        with (
            tc.tile_pool(name="const", bufs=1) as cpool,
            tc.tile_pool(name="kt8", bufs=1) as ktp,
            tc.tile_pool(name="v8", bufs=1) as vp,
            tc.tile_pool(name="qt", bufs=1) as qtp,
            tc.tile_pool(name="kbt", bufs=1) as kbtp,
            tc.tile_pool(name="vb", bufs=1) as vbp,
            tc.tile_pool(name="et", bufs=4) as etp,
            tc.tile_pool(name="ml", bufs=4) as mlp,
            tc.tile_pool(name="vec", bufs=10) as vecp,
            tc.tile_pool(name="out32", bufs=4) as o32p,
            tc.tile_pool(name="out16", bufs=3) as o16p,
            tc.tile_pool(name="poc", bufs=2, space="PSUM") as poc,
            tc.tile_pool(name="pst", bufs=3, space="PSUM") as pst,
            tc.tile_pool(name="psm", bufs=1, space="PSUM") as psm,
        ):
            # ---- constants & small tables (front of the DMA queue) ----
            ones8 = cpool.tile([128, 2, 2], F8)
            nc.sync.dma_start(ones8[:], ones8_in[:])
            ones16 = cpool.tile([128, 2], F16)
            nc.sync.dma_start(ones16[:], ones16_in[:])
            biasp = cpool.tile([128, NS, cfg.maxP], F32)
            nc.sync.dma_start(biasp[:], biasp_in[:])
            blend = cpool.tile([128, NS, 4], F32)
            nc.sync.dma_start(blend[:], blend_in[:])

            # warm the Exp table while input DMAs stream
            warm = vecp.tile([128, 2], F32, tag="v", name="warm")
            nc.scalar.activation(warm[:], ones16[:], AF.Exp, scale=1.0)

            # SHIFT bias as an AP (float biases need pre-registered consts)
            shift_t = cpool.tile([128, 1], F32)
            nc.gpsimd.memset(shift_t[:], SHIFT)

            # one persistent sums tile: slot j owns [:, j] (cross s, local s
            # chains at [:, j, s] / [:, j, 2+s]); zeroed once, all matmuls
            # accumulate with start=False so slots never clobber each other
            sums_all = psm.tile([128, NS, 4, 2], F32, name="sums_all")
            nc.vector.memset(sums_all[:], 0)

            # slot-0 fp8 queries come straight from the host
            qt8 = qtp.tile([128, NS, 4, 2, 256], F8)
            nc.sync.dma_start(qt8[:, 0], qt8s0_in[:])

            # resident K^T fp8, chunked so pair-0 matmuls start early
            kt8 = ktp.tile([128, KP, 2, 4, 2, 128], F8)
            kt_chunks = [(0, 1), (1, 2), (2, 4), (4, 8), (8, KP)]
            for c0, c1 in kt_chunks:
                if c0 < KP:
                    c1 = min(c1, KP)
                    nc.gpsimd.dma_start(kt8[:, c0:c1], kt8_in[:, c0:c1])
            # resident V fp8
            v8 = vp.tile([128, KP, 2, D], F8)
            for c0, c1 in kt_chunks:
                if c0 < KP:
                    c1 = min(c1, KP)
                    nc.gpsimd.dma_start(v8[:, c0:c1], v8_in[:, c0:c1])

            # fp16 per-slot tensors (slot-major chunks, slot 0 first)
            qt16 = qtp.tile([128, NS, 4, 2, 256], F16)
            kbt16 = kbtp.tile([128, NS, 4, 2, 2, 128], F16)
            vb16 = vbp.tile([128, NS, 2, D], F16)
            vb8 = vbp.tile([128, NS, 2, D], F8)
            for j in range(NS):
                nc.sync.dma_start(qt16[:, j], qt16_in[:, j])
                nc.sync.dma_start(kbt16[:, j], kbt16_in[:, j])
                nc.sync.dma_start(vb16[:, j], vb16_in[:, j])
                # on-chip casts: fp16 -> fp8 (saves DMA bytes)
                nc.gpsimd.tensor_copy(vb8[:, j], vb16[:, j])
                if j > 0:
                    nc.vector.tensor_copy(qt8[:, j], qt16[:, j])

            # ---- software-pipelined main loop over (slot, pair) ----
            pairs = [(j, t) for j in range(NS) for t in range(cfg.P[j])]
            LOOKAHEAD = 2
            state = {}       # (j, t) -> et2
            oc = {}          # slot -> [oc0, oc1]

            def emit_qk(j, t):
                st2 = pst.tile([128, 2, 256], F32, tag="st", name=f"st_{j}_{t}")
                for e in range(2):
                    for dp in range(4):
                        nc.tensor.matmul(
                            st2[:, e, :], kt8[:, t, e, dp, :, :],
                            qt8[:, j, dp, :, :],
                            start=(e == 0 and dp == 0),
                            stop=(e == 1 and dp == 3),
                            perf_mode=DR, skip_group_check=True)
                et2 = etp.tile([128, 2, 256], F8, tag="et", name=f"et_{j}_{t}")
                nc.scalar.activation(et2[:], st2[:], AF.Exp,
                                     bias=biasp[:, j, t:t + 1], scale=SCALE)
                state[(j, t)] = et2

            def emit_pv(j, t):
                et2 = state.pop((j, t))
                if t == 0:
                    oc[j] = [poc.tile([128, D], F32, tag="oc",
                                      name=f"oc{s}_{j}") for s in range(2)]
                for s in range(2):
                    lhs = et2[:, :, s * 128:(s + 1) * 128]
                    for dh in range(2):
                        nc.tensor.matmul(
                            oc[j][s][:, dh * 512:(dh + 1) * 512], lhs,
                            v8[:, t, :, dh * 512:(dh + 1) * 512],
                            start=(t == 0), stop=False,
                            perf_mode=DR, skip_group_check=True)
                    nc.tensor.matmul(sums_all[:, j, s, :], lhs, ones8[:],
                                     start=False, stop=False,
                                     perf_mode=DR, skip_group_check=True)

            def emit_boundary(j):
                # boundary scores fp16 in one score buffer:
                # [:, 0, :] = b0 keys x q 0:256, [:, 1, 0:128] = b1 x q 128:256
                stb = pst.tile([128, 2, 256], F32, tag="st", name=f"stb_{j}")
                for dp in range(4):
                    for u in range(2):
                        nc.tensor.matmul(
                            stb[:, 0, :], kbt16[:, j, dp, u, 0, :],
                            qt16[:, j, dp, u, :],
                            start=(dp == 0 and u == 0), stop=False,
                            skip_group_check=True)
                for dp in range(4):
                    for u in range(2):
                        nc.tensor.matmul(
                            stb[:, 1, 0:128], kbt16[:, j, dp, u, 1, :],
                            qt16[:, j, dp, u, 128:256],
                            start=False, stop=(dp == 3 and u == 1),
                            skip_group_check=True)

                # local fp16 masked exp tiles (diag blocks; rest zero)
                mls = []
                for s in range(2):
                    ml = mlp.tile([128, 128], F16, tag="ml", name=f"ml{s}_{j}")
                    nc.gpsimd.memset(ml[:], 0)
                    # stb[:,0] holds q 0:256; stb[:,1] holds q 128:256 at 0:128
                    c0 = 0
                    nc.scalar.activation(ml[0:64, 0:64],
                                         stb[0:64, s, c0:c0 + 64],
                                         AF.Exp, scale=SCALE)
                    nc.scalar.activation(ml[64:128, 64:128],
                                         stb[64:128, s, c0 + 64:c0 + 128],
                                         AF.Exp, scale=SCALE)
                    mls.append(ml)

                # masked fp8 exp rectangles for the boundary cross pair
                et2b = etp.tile([128, 2, 256], F8, tag="et", name=f"etb_{j}")
                nc.gpsimd.memset(et2b[:], 0)
                nc.scalar.activation(et2b[0:64, 0, 64:256],
                                     stb[0:64, 0, 64:256], AF.Exp,
                                     bias=shift_t[0:64], scale=SCALE)
                nc.scalar.activation(et2b[64:128, 0, 128:256],
                                     stb[64:128, 0, 128:256], AF.Exp,
                                     bias=shift_t[64:128], scale=SCALE)
                nc.scalar.activation(et2b[0:64, 1, 192:256],
                                     stb[0:64, 1, 64:128], AF.Exp,
                                     bias=shift_t[0:64], scale=SCALE)

                # fold boundary cross into oc / sums as one more DR pair
                for s in range(2):
                    lhs = et2b[:, :, s * 128:(s + 1) * 128]
                    for dh in range(2):
                        nc.tensor.matmul(
                            oc[j][s][:, dh * 512:(dh + 1) * 512], lhs,
                            vb8[:, j, :, dh * 512:(dh + 1) * 512],
                            start=False, stop=True,
                            perf_mode=DR, skip_group_check=True)
                    nc.tensor.matmul(sums_all[:, j, s, :], lhs, ones8[:],
                                     start=False, stop=(s == 1),
                                     perf_mode=DR, skip_group_check=True)

                # per-sub: cross flush, local PV (L reuses the oc buffer),
                # combine, out
                for s in range(2):
                    scm = vecp.tile([128, 1], F32, tag="v")
                    nc.vector.tensor_scalar_max(scm[:],
                                                sums_all[:, j, s, 0:1], 1e-30)
                    rc = vecp.tile([128, 1], F32, tag="v")
                    nc.vector.reciprocal(rc[:], scm[:])
                    rc2 = vecp.tile([128, 1], F32, tag="v")
                    nc.vector.tensor_mul(rc2[:], rc[:],
                                         blend[:, j, 2 * s + 1:2 * s + 2])
                    cs = o32p.tile([128, D], F32, tag="o32", name=f"cs{s}_{j}")
                    nc.scalar.activation(cs[:], oc[j][s][:], AF.Copy,
                                         scale=rc2[:])

                    # local sums (same shared bank, start=False accumulate)
                    nc.tensor.matmul(sums_all[:, j, 2 + s, :], mls[s][:],
                                     ones16[:], start=False, stop=True,
                                     skip_group_check=True)
                    slm = vecp.tile([128, 1], F32, tag="v")
                    nc.vector.tensor_scalar_max(slm[:],
                                                sums_all[:, j, 2 + s, 0:1],
                                                1e-30)
                    rl = vecp.tile([128, 1], F32, tag="v")
                    nc.vector.reciprocal(rl[:], slm[:])
                    rl2 = vecp.tile([128, 1], F32, tag="v")
                    nc.vector.tensor_mul(rl2[:], rl[:],
                                         blend[:, j, 2 * s:2 * s + 1])

                    L = poc.tile([128, D], F32, tag="oc", name=f"L{s}_{j}")
                    for dh in range(2):
                        nc.tensor.matmul(
                            L[:, dh * 512:(dh + 1) * 512], mls[s][:],
                            vb16[:, j, s, dh * 512:(dh + 1) * 512],
                            start=True, stop=True)
                    fin16 = o16p.tile([128, D], F16, tag="o16",
                                      name=f"fin{s}_{j}")
                    for dh in range(2):
                        lt = o32p.tile([128, 512], F32, tag="o32",
                                       name=f"lt{s}{dh}_{j}")
                        nc.vector.tensor_scalar_mul(
                            lt[:], L[:, dh * 512:(dh + 1) * 512], rl2[:])
                        nc.gpsimd.tensor_add(
                            fin16[:, dh * 512:(dh + 1) * 512], lt[:],
                            cs[:, dh * 512:(dh + 1) * 512])
                    row = (2 * j + s) * 128
                    nc.sync.dma_start(out_t[row:row + 128, :], fin16[:])

            qk_idx = 0
            for i, (j, t) in enumerate(pairs):
                while qk_idx <= i + LOOKAHEAD and qk_idx < len(pairs):
                    emit_qk(*pairs[qk_idx])
                    qk_idx += 1
                emit_pv(j, t)
                if t == cfg.P[j] - 1:
                    emit_boundary(j)
    nc.compile()
    return nc


def _host_inputs(cfg: Cfg, query, key, value):
    """Build per-core input maps (fp8/fp16 pre-layouts). K/V layouts are
    shared per batch; per-core tensors are group gathers."""
    D = cfg.D
    NS = cfg.n_slot
    KP = min(cfg.maxP, cfg.npair)
    B = query.shape[0]

    kv_cache = []
    for b in range(B):
        # kt8[p, t, e, dp, u, kk] = K[(2t+e)*128+kk, dp*256+u*128+p]
        k8 = key[b, :KP * 256].astype(NPF8)          # [S', D]
        k8 = k8.reshape(KP, 2, 128, 4, 2, 128)       # [t, e, kk, dp, u, p]
        kt8 = np.ascontiguousarray(k8.transpose(5, 0, 1, 3, 4, 2))
        v8 = value[b, :KP * 256].astype(NPF8).reshape(KP, 2, 128, D)
        v8 = np.ascontiguousarray(v8.transpose(2, 0, 1, 3))  # [p, t, e, d]
        kv_cache.append((kt8, v8))

    ones8 = np.ones((128, 2, 2), NPF8)
    ones16 = np.ones((128, 2), np.float16)

    in_maps = []
    for core in range(cfg.n_cores):
        b = core // cfg.cores_per_batch
        qt_idx = core % cfg.cores_per_batch
        groups = [cls[qt_idx] for cls in cfg.classes]
        kt8, v8 = kv_cache[b]

        # [NS, 256, D] row gathers
        q_rows = np.stack([query[b, g * 256:(g + 1) * 256] for g in groups])
        k_rows = np.stack([key[b, g * 256:(g + 1) * 256] for g in groups])
        v_rows = np.stack([value[b, g * 256:(g + 1) * 256] for g in groups])

        # qt16[p, j, dp, u, q] = Q[j, q, dp*256+u*128+p]
        qt = q_rows.astype(np.float16).reshape(NS, 256, 4, 2, 128)
        qt16 = np.ascontiguousarray(qt.transpose(4, 0, 2, 3, 1))
        qt8s0 = np.ascontiguousarray(qt16[:, 0]).astype(NPF8)
        # kbt16[p, j, dp, u, blk, kk] = K[j, blk*128+kk, dp*256+u*128+p]
        kb = k_rows.astype(np.float16).reshape(NS, 2, 128, 4, 2, 128)
        kbt16 = np.ascontiguousarray(kb.transpose(5, 0, 3, 4, 1, 2))
        # vb16[p, j, e, d]
        vb = v_rows.astype(np.float16).reshape(NS, 2, 128, D)
        vb16 = np.ascontiguousarray(vb.transpose(2, 0, 1, 3))

        biasp = np.full((128, NS, cfg.maxP), NEG, np.float32)
        blendt = np.zeros((128, NS, 4), np.float32)
        for j, g in enumerate(groups):
            biasp[:, j, :g] = SHIFT
            for s in range(2):
                for half in range(2):
                    chunk = 4 * g + 2 * s + half
                    sl = slice(half * 64, half * 64 + 64)
                    blendt[sl, j, 2 * s] = 1.0 if chunk == 0 else 0.9
                    blendt[sl, j, 2 * s + 1] = 0.0 if chunk == 0 else 0.1

        in_maps.append({
            "kt8_in": kt8, "v8_in": v8,
            "qt16_in": qt16, "qt8s0_in": qt8s0,
            "kbt16_in": kbt16, "vb16_in": vb16,
            "biasp_in": biasp, "blend_in": blendt,
            "ones8_in": ones8, "ones16_in": ones16,
        })
    return in_maps


def _scatter_output(cfg: Cfg, results, B):
    out = np.empty((B, cfg.S, cfg.D), np.float32)
    for core in range(cfg.n_cores):
        b = core // cfg.cores_per_batch
        qt_idx = core % cfg.cores_per_batch
        groups = [cls[qt_idx] for cls in cfg.classes]
        oc = np.asarray(results[core]["out_core"], np.float32)
        for j, g in enumerate(groups):
            out[b, g * cfg.GQ:(g + 1) * cfg.GQ] = oc[j * cfg.GQ:(j + 1) * cfg.GQ]
    return out


_nc_cache = {}


def run(cfg: Cfg, query, key, value, trace=False, trace_kwargs=None):
    ck = cfg.S
    if ck not in _nc_cache:
        _nc_cache[ck] = build_nc(cfg)
    nc = _nc_cache[ck]
    in_maps = _host_inputs(cfg, query, key, value)
    kw = {}
    if trace:
        kw = dict(trace=True, trace_cores=list(range(cfg.n_cores)),
                  **(trace_kwargs or {}))
    res = run_bass_kernel_spmd(nc, in_maps, core_ids=list(range(cfg.n_cores)),
                               **kw)
    out = _scatter_output(cfg, res.results, query.shape[0])
    return out, res


def kernel(query, key, value):
    query = np.asarray(query, np.float32)
    key = np.asarray(key, np.float32)
    value = np.asarray(value, np.float32)
    out, _ = run(FULL, query, key, value)
    return out


# revision 15
# speedup vs baseline: 2.7375x; 1.3595x over previous
"""ChunkedSparseAttention Trainium2 kernel (fp8/fp16 rewrite).

Problem: B=2, S=4096, D=1024, CHUNK=64. Per chunk i:
  local  = softmax(Qi @ Ki^T / 32) @ Vi            (own 64 keys)
  cross  = softmax(Qi @ K[:64i]^T / 32) @ V[:64i]  (prefix keys)
  out_i  = local                     if i == 0
         = 0.9 * local + 0.1 * cross otherwise

Distribution (same as the f32r baseline): 8 cores, data-parallel over batch
(4 cores/batch); each core takes one 4-chunk group (256 queries) from each of
4 classes ({0-3},{4-7},{8-11},{12-15}) so triangular prefix work is balanced.
One SPMD NEFF: per-class key-block loops padded to the class max; padded
key-block PAIRS are masked via a per-core bias table (exp(s/32 + b), b=-1e9
kills a padded pair, b=-2 on real pairs keeps exp() within fp8e4 range — the
shift cancels in the softmax ratio).

Precision strategy (correctness gate is rel_err < 2e-2):
  - cross/prefix path (alpha=0.1): fp8e4 everywhere with DoubleRow matmuls
    (2 key-blocks contracted per instruction; QK contracts d in 4 insts/kb,
    PV contracts 256 keys/inst). K^T and V live in SBUF in fp8 (3.75MB each).
  - local path (weight 0.9) + boundary scores: fp16 (1 cycle/row like bf16
    but 8x the mantissa). Local PV uses masked fp16 exp tiles (diag blocks,
    off-diag zeroed) so the matmuls run with the full 128-partition dim.
  - output: fp16 (halves the out DMA; 4.9e-4 rel rounding).
The boundary cross pieces are folded into the main cross accumulators as one
extra DoubleRow pair per slot (masked exp rectangles), removing the baseline's
fp32 bitcast boundary matmuls entirely.

PSUM (8 banks): oc[2 subs] 4 banks + score pipeline 3 banks + sums 1 bank.
The sums bank holds 4 accumulation chains (cross/local x 2 subs) using the
PSUM zero-region semantics: only the very first matmul uses start=True, later
chains start=False and read-as-zero (validated on this stack).
"""
import sys

for _p in ("/opt/trn_rl_repo", "/root/.axon_site/_ro/trn_rl_repo"):
    if _p not in sys.path:
        sys.path.insert(0, _p)

import numpy as np
import ml_dtypes

import concourse.bass as bass
import concourse.mybir as mybir
import concourse.tile as tile
from concourse import bacc
from concourse.bass_utils import run_bass_kernel_spmd

F32 = mybir.dt.float32
F16 = mybir.dt.float16
F8 = mybir.dt.float8e4
AF = mybir.ActivationFunctionType
DR = mybir.MatmulPerfMode.DoubleRow
NPF8 = ml_dtypes.float8_e4m3
SCALE = 1.0 / 32.0  # 1/sqrt(D)
NEG = -1e9
SHIFT = -2.0        # exp shift on the cross path; cancels in softmax


class Cfg:
    def __init__(self, S, classes):
        self.S = S
        self.D = 1024
        self.classes = classes              # 4 lists of group indices
        self.n_slot = len(classes)
        self.P = [max(max(c), 1) for c in classes]  # padded PAIR count / slot
        self.maxP = max(self.P)
        self.npair = self.S // 256          # total key pairs in sequence
        self.GQ = 256                       # queries per group (4 chunks)
        self.NQ = self.n_slot * self.GQ     # queries per core
        self.cores_per_batch = len(classes[0])
        self.n_cores = 2 * self.cores_per_batch
        # slot processing order: descending pair count
        self.js = sorted(range(self.n_slot), key=lambda j: -self.P[j])


FULL = Cfg(4096, [[0, 1, 2, 3], [4, 5, 6, 7], [8, 9, 10, 11], [12, 13, 14, 15]])
MINI = Cfg(1024, [[0], [1], [2], [3]])


def build_nc(cfg: Cfg):
    D = cfg.D
    NS = cfg.n_slot
    KP = min(cfg.maxP, cfg.npair)           # resident K/V pair count
    nc = bacc.Bacc("TRN2", target_bir_lowering=False, debug=False)

    # ---- external inputs (host pre-laid-out, partition-major) ----
    # kt8[p, t, e, dp, u, kk] = K[(2t+e)*128+kk, dp*256+u*128+p]
    kt8_in = nc.dram_tensor("kt8_in", [128, KP, 2, 4, 2, 128], F8,
                            kind="ExternalInput")
    # v8[p, t, e, d] = V[(2t+e)*128+p, d]
    v8_in = nc.dram_tensor("v8_in", [128, KP, 2, D], F8, kind="ExternalInput")
    # qt16[p, j, dp, u, q] = Q[group_j q, dp*256+u*128+p]
    qt16_in = nc.dram_tensor("qt16_in", [128, NS, 4, 2, 256], F16,
                             kind="ExternalInput")
    # qt8 for slot 0 only (avoids gating PE start on the DVE cast)
    qt8s0_in = nc.dram_tensor("qt8s0_in", [128, 4, 2, 256], F8,
                              kind="ExternalInput")
    # kbt16[p, j, dp, u, blk, kk] = K[group_j key blk*128+kk, dp*256+u*128+p]
    kbt16_in = nc.dram_tensor("kbt16_in", [128, NS, 4, 2, 2, 128], F16,
                              kind="ExternalInput")
    # vb16[p, j, e, d] = V[group_j key e*128+p, d]
    vb16_in = nc.dram_tensor("vb16_in", [128, NS, 2, D], F16,
                             kind="ExternalInput")
    # biasp[p, j, t]: -2 (real pair) / -1e9 (padded pair)
    biasp_in = nc.dram_tensor("biasp_in", [128, NS, cfg.maxP], F32,
                              kind="ExternalInput")
    # blend[p, j, 2s]=local coeff, [p, j, 2s+1]=alpha
    blend_in = nc.dram_tensor("blend_in", [128, NS, 4], F32,
                              kind="ExternalInput")
    ones8_in = nc.dram_tensor("ones8_in", [128, 2, 2], F8, kind="ExternalInput")
    ones16_in = nc.dram_tensor("ones16_in", [128, 2], F16, kind="ExternalInput")
    out_t = nc.dram_tensor("out_core", [cfg.NQ, D], F16, kind="ExternalOutput")

    with tile.TileContext(nc) as tc:
        with (
            tc.tile_pool(name="const", bufs=1) as cpool,
            tc.tile_pool(name="kt8", bufs=1) as ktp,
            tc.tile_pool(name="v8", bufs=1) as vp,
            tc.tile_pool(name="qt", bufs=1) as qtp,
            tc.tile_pool(name="kbt", bufs=1) as kbtp,
            tc.tile_pool(name="vb", bufs=1) as vbp,
            tc.tile_pool(name="et", bufs=8) as etp,
            tc.tile_pool(name="ml", bufs=8) as mlp,
            tc.tile_pool(name="vec", bufs=10) as vecp,
            tc.tile_pool(name="out32", bufs=4) as o32p,
            tc.tile_pool(name="out16", bufs=3) as o16p,
            tc.tile_pool(name="poc", bufs=2, space="PSUM") as poc,
            tc.tile_pool(name="pst", bufs=3, space="PSUM") as pst,
            tc.tile_pool(name="psm", bufs=1, space="PSUM") as psm,
        ):
            # ---- constants & small tables (front of the DMA queue) ----
            ones8 = cpool.tile([128, 2, 2], F8)
            nc.scalar.dma_start(ones8[:], ones8_in[:])
            ones16 = cpool.tile([128, 2], F16)
            nc.scalar.dma_start(ones16[:], ones16_in[:])
            biasp = cpool.tile([128, NS, cfg.maxP], F32)
            nc.scalar.dma_start(biasp[:], biasp_in[:])
            blend = cpool.tile([128, NS, 4], F32)
            nc.scalar.dma_start(blend[:], blend_in[:])

            # warm the Exp table while input DMAs stream
            warm = vecp.tile([128, 2], F32, tag="v", name="warm")
            nc.scalar.activation(warm[:], ones16[:], AF.Exp, scale=1.0)

            # SHIFT bias as an AP (float biases need pre-registered consts)
            shift_t = cpool.tile([128, 1], F32)
            nc.gpsimd.memset(shift_t[:], SHIFT)

            # one persistent sums tile: slot j owns [:, j] (cross s, local s
            # chains at [:, j, s] / [:, j, 2+s]); zeroed once, all matmuls
            # accumulate with start=False so slots never clobber each other
            sums_all = psm.tile([128, NS, 4, 2], F32, name="sums_all")
            nc.vector.memset(sums_all[:], 0)

            # slots processed in DESCENDING pair count: the big slot
            # absorbs the K/V streaming phase, later slots run resident
            js = cfg.js

            # first-processed slot's fp8 queries come straight from the host
            qt8 = qtp.tile([128, NS, 4, 2, 256], F8)
            nc.sync.dma_start(qt8[:, js[0]], qt8s0_in[:])

            kt8 = ktp.tile([128, KP, 2, 4, 2, 128], F8)
            v8 = vp.tile([128, KP, 2, D], F8)
            qt16 = qtp.tile([128, NS, 4, 2, 256], F16)
            kbt16 = kbtp.tile([128, NS, 4, 2, 2, 128], F16)
            vb16 = vbp.tile([128, NS, 2, D], F16)
            vb8 = vbp.tile([128, NS, 2, D], F8)

            # pair 0 on the fast HWDGE queue so the PE starts early
            nc.sync.dma_start(kt8[:, 0:1], kt8_in[:, 0:1])
            nc.sync.dma_start(v8[:, 0:1], v8_in[:, 0:1])

            # single SWDGE queue in exact need-order: K/V pair chunks
            # interleaved with per-slot boundary tensors (descending slots)
            kv_chunks = [(1, 3), (3, 6), (6, 9), (9, 12), (12, KP)]
            kv_chunks = [(a, min(b, KP)) for a, b in kv_chunks if a < KP]
            for c0, c1 in kv_chunks:
                nc.gpsimd.dma_start(kt8[:, c0:c1], kt8_in[:, c0:c1])
                nc.gpsimd.dma_start(v8[:, c0:c1], v8_in[:, c0:c1])
            for j in js:
                nc.gpsimd.dma_start(qt16[:, j], qt16_in[:, j])
                nc.gpsimd.dma_start(kbt16[:, j], kbt16_in[:, j])
                nc.gpsimd.dma_start(vb16[:, j], vb16_in[:, j])
                # on-chip casts (DVE): fp16 -> fp8
                nc.vector.tensor_copy(vb8[:, j], vb16[:, j])
                if j != js[0]:
                    nc.vector.tensor_copy(qt8[:, j], qt16[:, j])

            # ---- software-pipelined main loop over (slot, pair) ----
            pairs = [(j, t) for j in js for t in range(cfg.P[j])]
            LOOKAHEAD = 2
            state = {}       # (j, t) -> et2
            oc = {}          # slot -> [oc0, oc1]
            bnd = {}         # slot -> (mls, et2b) from hoisted boundary work

            def emit_qk(j, t):
                st2 = pst.tile([128, 2, 256], F32, tag="st", name=f"st_{j}_{t}")
                for e in range(2):
                    for dp in range(4):
                        nc.tensor.matmul(
                            st2[:, e, :], kt8[:, t, e, dp, :, :],
                            qt8[:, j, dp, :, :],
                            start=(e == 0 and dp == 0),
                            stop=(e == 1 and dp == 3),
                            perf_mode=DR, skip_group_check=True)
                et2 = etp.tile([128, 2, 256], F8, tag="et", name=f"et_{j}_{t}")
                nc.scalar.activation(et2[:], st2[:], AF.Exp,
                                     bias=biasp[:, j, t:t + 1], scale=SCALE)
                state[(j, t)] = et2

            def emit_pv(j, t):
                et2 = state.pop((j, t))
                if t == 0:
                    oc[j] = [poc.tile([128, D], F32, tag="oc",
                                      name=f"oc{s}_{j}") for s in range(2)]
                for s in range(2):
                    lhs = et2[:, :, s * 128:(s + 1) * 128]
                    for dh in range(2):
                        nc.tensor.matmul(
                            oc[j][s][:, dh * 512:(dh + 1) * 512], lhs,
                            v8[:, t, :, dh * 512:(dh + 1) * 512],
                            start=(t == 0), stop=False,
                            perf_mode=DR, skip_group_check=True)
                    nc.tensor.matmul(sums_all[:, j, s, :], lhs, ones8[:],
                                     start=False, stop=False,
                                     perf_mode=DR, skip_group_check=True)

            def emit_bqk(j):
                """Boundary scores + masked exp tiles + local sums. Needs only
                kbt16/qt16 — hoisted into the first slot's pair stream as
                stall filler while K/V stream from HBM."""
                # boundary scores fp16 in one score buffer:
                # [:, 0, :] = b0 keys x q 0:256, [:, 1, 0:128] = b1 x q 128:256
                stb = pst.tile([128, 2, 256], F32, tag="st", name=f"stb_{j}")
                for dp in range(4):
                    for u in range(2):
                        nc.tensor.matmul(
                            stb[:, 0, :], kbt16[:, j, dp, u, 0, :],
                            qt16[:, j, dp, u, :],
                            start=(dp == 0 and u == 0), stop=False,
                            skip_group_check=True)
                for dp in range(4):
                    for u in range(2):
                        nc.tensor.matmul(
                            stb[:, 1, 0:128], kbt16[:, j, dp, u, 1, :],
                            qt16[:, j, dp, u, 128:256],
                            start=False, stop=(dp == 3 and u == 1),
                            skip_group_check=True)

                # local fp16 masked exp tiles (diag blocks; rest zero)
                mls = []
                for s in range(2):
                    ml = mlp.tile([128, 128], F16, tag="ml", name=f"ml{s}_{j}")
                    nc.gpsimd.memset(ml[:], 0)
                    # stb[:,0] holds q 0:256; stb[:,1] holds q 128:256 at 0:128
                    c0 = 0
                    nc.scalar.activation(ml[0:64, 0:64],
                                         stb[0:64, s, c0:c0 + 64],
                                         AF.Exp, scale=SCALE)
                    nc.scalar.activation(ml[64:128, 64:128],
                                         stb[64:128, s, c0 + 64:c0 + 128],
                                         AF.Exp, scale=SCALE)
                    mls.append(ml)

                # masked fp8 exp rectangles for the boundary cross pair
                et2b = etp.tile([128, 2, 256], F8, tag="et", name=f"etb_{j}")
                nc.gpsimd.memset(et2b[:], 0)
                nc.scalar.activation(et2b[0:64, 0, 64:256],
                                     stb[0:64, 0, 64:256], AF.Exp,
                                     bias=shift_t[0:64], scale=SCALE)
                nc.scalar.activation(et2b[64:128, 0, 128:256],
                                     stb[64:128, 0, 128:256], AF.Exp,
                                     bias=shift_t[64:128], scale=SCALE)
                nc.scalar.activation(et2b[0:64, 1, 192:256],
                                     stb[0:64, 1, 64:128], AF.Exp,
                                     bias=shift_t[0:64], scale=SCALE)

                # local sums (shared-bank region, start=False accumulate)
                for s in range(2):
                    nc.tensor.matmul(sums_all[:, j, 2 + s, :], mls[s][:],
                                     ones16[:], start=False, stop=True,
                                     skip_group_check=True)
                bnd[j] = (mls, et2b)

            def emit_tail(j):
                mls, et2b = bnd.pop(j)
                # fold boundary cross into oc / sums as one more DR pair
                for s in range(2):
                    lhs = et2b[:, :, s * 128:(s + 1) * 128]
                    for dh in range(2):
                        nc.tensor.matmul(
                            oc[j][s][:, dh * 512:(dh + 1) * 512], lhs,
                            vb8[:, j, :, dh * 512:(dh + 1) * 512],
                            start=False, stop=True,
                            perf_mode=DR, skip_group_check=True)
                    nc.tensor.matmul(sums_all[:, j, s, :], lhs, ones8[:],
                                     start=False, stop=(s == 1),
                                     perf_mode=DR, skip_group_check=True)

                # per-sub: cross flush, local PV (L reuses the oc buffer),
                # combine, out
                for s in range(2):
                    scm = vecp.tile([128, 1], F32, tag="v")
                    nc.vector.tensor_scalar_max(scm[:],
                                                sums_all[:, j, s, 0:1], 1e-30)
                    rc = vecp.tile([128, 1], F32, tag="v")
                    nc.vector.reciprocal(rc[:], scm[:])
                    rc2 = vecp.tile([128, 1], F32, tag="v")
                    nc.vector.tensor_mul(rc2[:], rc[:],
                                         blend[:, j, 2 * s + 1:2 * s + 2])
                    cs = o32p.tile([128, D], F32, tag="o32", name=f"cs{s}_{j}")
                    nc.scalar.activation(cs[:], oc[j][s][:], AF.Copy,
                                         scale=rc2[:])

                    slm = vecp.tile([128, 1], F32, tag="v")
                    nc.vector.tensor_scalar_max(slm[:],
                                                sums_all[:, j, 2 + s, 0:1],
                                                1e-30)
                    rl = vecp.tile([128, 1], F32, tag="v")
                    nc.vector.reciprocal(rl[:], slm[:])
                    rl2 = vecp.tile([128, 1], F32, tag="v")
                    nc.vector.tensor_mul(rl2[:], rl[:],
                                         blend[:, j, 2 * s:2 * s + 1])

                    L = poc.tile([128, D], F32, tag="oc", name=f"L{s}_{j}")
                    for dh in range(2):
                        nc.tensor.matmul(
                            L[:, dh * 512:(dh + 1) * 512], mls[s][:],
                            vb16[:, j, s, dh * 512:(dh + 1) * 512],
                            start=True, stop=True)
                    fin16 = o16p.tile([128, D], F16, tag="o16",
                                      name=f"fin{s}_{j}")
                    for dh in range(2):
                        lt = o32p.tile([128, 512], F32, tag="o32",
                                       name=f"lt{s}{dh}_{j}")
                        nc.vector.tensor_scalar_mul(
                            lt[:], L[:, dh * 512:(dh + 1) * 512], rl2[:])
                        nc.gpsimd.tensor_add(
                            fin16[:, dh * 512:(dh + 1) * 512], lt[:],
                            cs[:, dh * 512:(dh + 1) * 512])
                    row = (2 * j + s) * 128
                    nc.sync.dma_start(out_t[row:row + 128, :], fin16[:])

            # boundary precompute insertion points: spread across the first
            # (largest) slot's pair stream, but never after the slot's tail
            deadline = {}
            for i, (j, t) in enumerate(pairs):
                if t == cfg.P[j] - 1:
                    deadline[j] = i
            ins = {}
            for j in js:
                ins.setdefault(max(deadline[j] - 2, 0), []).append(j)

            qk_idx = 0
            for i, (j, t) in enumerate(pairs):
                for jb in ins.get(i, []):
                    emit_bqk(jb)
                while qk_idx <= i + LOOKAHEAD and qk_idx < len(pairs):
                    emit_qk(*pairs[qk_idx])
                    qk_idx += 1
                emit_pv(j, t)
                if t == cfg.P[j] - 1:
                    emit_tail(j)
    nc.compile()
    return nc


def _host_inputs(cfg: Cfg, query, key, value):
    """Build per-core input maps (fp8/fp16 pre-layouts). K/V layouts are
    shared per batch; per-core tensors are group gathers."""
    D = cfg.D
    NS = cfg.n_slot
    KP = min(cfg.maxP, cfg.npair)
    B = query.shape[0]

    kv_cache = []
    for b in range(B):
        # kt8[p, t, e, dp, u, kk] = K[(2t+e)*128+kk, dp*256+u*128+p]
        k8 = key[b, :KP * 256].astype(NPF8)          # [S', D]
        k8 = k8.reshape(KP, 2, 128, 4, 2, 128)       # [t, e, kk, dp, u, p]
        kt8 = np.ascontiguousarray(k8.transpose(5, 0, 1, 3, 4, 2))
        v8 = value[b, :KP * 256].astype(NPF8).reshape(KP, 2, 128, D)
        v8 = np.ascontiguousarray(v8.transpose(2, 0, 1, 3))  # [p, t, e, d]
        kv_cache.append((kt8, v8))

    ones8 = np.ones((128, 2, 2), NPF8)
    ones16 = np.ones((128, 2), np.float16)

    in_maps = []
    for core in range(cfg.n_cores):
        b = core // cfg.cores_per_batch
        qt_idx = core % cfg.cores_per_batch
        groups = [cls[qt_idx] for cls in cfg.classes]
        kt8, v8 = kv_cache[b]

        # [NS, 256, D] row gathers
        q_rows = np.stack([query[b, g * 256:(g + 1) * 256] for g in groups])
        k_rows = np.stack([key[b, g * 256:(g + 1) * 256] for g in groups])
        v_rows = np.stack([value[b, g * 256:(g + 1) * 256] for g in groups])

        # qt16[p, j, dp, u, q] = Q[j, q, dp*256+u*128+p]
        qt = q_rows.astype(np.float16).reshape(NS, 256, 4, 2, 128)
        qt16 = np.ascontiguousarray(qt.transpose(4, 0, 2, 3, 1))
        qt8s0 = np.ascontiguousarray(qt16[:, cfg.js[0]]).astype(NPF8)
        # kbt16[p, j, dp, u, blk, kk] = K[j, blk*128+kk, dp*256+u*128+p]
        kb = k_rows.astype(np.float16).reshape(NS, 2, 128, 4, 2, 128)
        kbt16 = np.ascontiguousarray(kb.transpose(5, 0, 3, 4, 1, 2))
        # vb16[p, j, e, d]
        vb = v_rows.astype(np.float16).reshape(NS, 2, 128, D)
        vb16 = np.ascontiguousarray(vb.transpose(2, 0, 1, 3))

        biasp = np.full((128, NS, cfg.maxP), NEG, np.float32)
        blendt = np.zeros((128, NS, 4), np.float32)
        for j, g in enumerate(groups):
            biasp[:, j, :g] = SHIFT
            for s in range(2):
                for half in range(2):
                    chunk = 4 * g + 2 * s + half
                    sl = slice(half * 64, half * 64 + 64)
                    blendt[sl, j, 2 * s] = 1.0 if chunk == 0 else 0.9
                    blendt[sl, j, 2 * s + 1] = 0.0 if chunk == 0 else 0.1

        in_maps.append({
            "kt8_in": kt8, "v8_in": v8,
            "qt16_in": qt16, "qt8s0_in": qt8s0,
            "kbt16_in": kbt16, "vb16_in": vb16,
            "biasp_in": biasp, "blend_in": blendt,
            "ones8_in": ones8, "ones16_in": ones16,
        })
    return in_maps


def _scatter_output(cfg: Cfg, results, B):
    out = np.empty((B, cfg.S, cfg.D), np.float32)
    for core in range(cfg.n_cores):
        b = core // cfg.cores_per_batch
        qt_idx = core % cfg.cores_per_batch
        groups = [cls[qt_idx] for cls in cfg.classes]
        oc = np.asarray(results[core]["out_core"], np.float32)
        for j, g in enumerate(groups):
            out[b, g * cfg.GQ:(g + 1) * cfg.GQ] = oc[j * cfg.GQ:(j + 1) * cfg.GQ]
    return out


_nc_cache = {}


def run(cfg: Cfg, query, key, value, trace=False, trace_kwargs=None):
    ck = cfg.S
    if ck not in _nc_cache:
        _nc_cache[ck] = build_nc(cfg)
    nc = _nc_cache[ck]
    in_maps = _host_inputs(cfg, query, key, value)
    kw = {}
    if trace:
        kw = dict(trace=True, trace_cores=list(range(cfg.n_cores)),
                  **(trace_kwargs or {}))
    res = run_bass_kernel_spmd(nc, in_maps, core_ids=list(range(cfg.n_cores)),
                               **kw)
    out = _scatter_output(cfg, res.results, query.shape[0])
    return out, res


def kernel(query, key, value):
    query = np.asarray(query, np.float32)
    key = np.asarray(key, np.float32)
    value = np.asarray(value, np.float32)
    out, _ = run(FULL, query, key, value)
    return out


# revision 17
# speedup vs baseline: 2.9419x; 1.0747x over previous
"""ChunkedSparseAttention Trainium2 kernel (fp8/fp16 rewrite).

Problem: B=2, S=4096, D=1024, CHUNK=64. Per chunk i:
  local  = softmax(Qi @ Ki^T / 32) @ Vi            (own 64 keys)
  cross  = softmax(Qi @ K[:64i]^T / 32) @ V[:64i]  (prefix keys)
  out_i  = local                     if i == 0
         = 0.9 * local + 0.1 * cross otherwise

Distribution (same as the f32r baseline): 8 cores, data-parallel over batch
(4 cores/batch); each core takes one 4-chunk group (256 queries) from each of
4 classes ({0-3},{4-7},{8-11},{12-15}) so triangular prefix work is balanced.
One SPMD NEFF: per-class key-block loops padded to the class max; padded
key-block PAIRS are masked via a per-core bias table (exp(s/32 + b), b=-1e9
kills a padded pair, b=-2 on real pairs keeps exp() within fp8e4 range — the
shift cancels in the softmax ratio).

Precision strategy (correctness gate is rel_err < 2e-2):
  - cross/prefix path (alpha=0.1): fp8e4 everywhere with DoubleRow matmuls
    (2 key-blocks contracted per instruction; QK contracts d in 4 insts/kb,
    PV contracts 256 keys/inst). K^T and V live in SBUF in fp8 (3.75MB each).
  - local path (weight 0.9) + boundary scores: fp16 (1 cycle/row like bf16
    but 8x the mantissa). Local PV uses masked fp16 exp tiles (diag blocks,
    off-diag zeroed) so the matmuls run with the full 128-partition dim.
  - output: fp16 (halves the out DMA; 4.9e-4 rel rounding).
The boundary cross pieces are folded into the main cross accumulators as one
extra DoubleRow pair per slot (masked exp rectangles), removing the baseline's
fp32 bitcast boundary matmuls entirely.

PSUM (8 banks): oc[2 subs] 4 banks + score pipeline 3 banks + sums 1 bank.
The sums bank holds 4 accumulation chains (cross/local x 2 subs) using the
PSUM zero-region semantics: only the very first matmul uses start=True, later
chains start=False and read-as-zero (validated on this stack).
"""
import sys

for _p in ("/opt/trn_rl_repo", "/root/.axon_site/_ro/trn_rl_repo"):
    if _p not in sys.path:
        sys.path.insert(0, _p)

import numpy as np
import ml_dtypes

import concourse.bass as bass
import concourse.mybir as mybir
import concourse.tile as tile
from concourse import bacc
from concourse.bass_utils import run_bass_kernel_spmd

F32 = mybir.dt.float32
F16 = mybir.dt.float16
F8 = mybir.dt.float8e4
AF = mybir.ActivationFunctionType
DR = mybir.MatmulPerfMode.DoubleRow
NPF8 = ml_dtypes.float8_e4m3
SCALE = 1.0 / 32.0  # 1/sqrt(D)
NEG = -1e9
SHIFT = -2.0        # exp shift on the cross path; cancels in softmax


class Cfg:
    def __init__(self, S, classes):
        self.S = S
        self.D = 1024
        self.classes = classes              # 4 lists of group indices
        self.n_slot = len(classes)
        self.P = [max(max(c), 1) for c in classes]  # padded PAIR count / slot
        self.maxP = max(self.P)
        self.npair = self.S // 256          # total key pairs in sequence
        self.GQ = 256                       # queries per group (4 chunks)
        self.NQ = self.n_slot * self.GQ     # queries per core
        self.cores_per_batch = len(classes[0])
        self.n_cores = 2 * self.cores_per_batch
        # slot processing order: descending pair count
        self.js = sorted(range(self.n_slot), key=lambda j: -self.P[j])


FULL = Cfg(4096, [[0, 1, 2, 3], [4, 5, 6, 7], [8, 9, 10, 11], [12, 13, 14, 15]])
MINI = Cfg(1024, [[0], [1], [2], [3]])


def build_nc(cfg: Cfg):
    D = cfg.D
    NS = cfg.n_slot
    KP = min(cfg.maxP, cfg.npair)           # resident K/V pair count
    nc = bacc.Bacc("TRN2", target_bir_lowering=False, debug=False)

    # ---- external inputs (host pre-laid-out, partition-major) ----
    # kt8[p, t, e, dp, u, kk] = K[(2t+e)*128+kk, dp*256+u*128+p]
    kt8_in = nc.dram_tensor("kt8_in", [128, KP, 2, 4, 2, 128], F8,
                            kind="ExternalInput")
    # v8[p, t, e, d] = V[(2t+e)*128+p, d]
    v8_in = nc.dram_tensor("v8_in", [128, KP, 2, D], F8, kind="ExternalInput")
    # qt16[p, j, dp, u, q] = Q[group_j q, dp*256+u*128+p]
    qt16_in = nc.dram_tensor("qt16_in", [128, NS, 4, 2, 256], F16,
                             kind="ExternalInput")
    # fp8 queries (cross path) loaded directly
    qt8_in = nc.dram_tensor("qt8_in", [128, NS, 4, 2, 256], F8,
                            kind="ExternalInput")
    # fp8 own-group V rows (boundary cross pair rhs) loaded directly
    vb8_in = nc.dram_tensor("vb8_in", [128, NS, 2, D], F8,
                            kind="ExternalInput")
    # kbt16[p, j, dp, u, blk, kk] = K[group_j key blk*128+kk, dp*256+u*128+p]
    kbt16_in = nc.dram_tensor("kbt16_in", [128, NS, 4, 2, 2, 128], F16,
                              kind="ExternalInput")
    # vb16[p, j, e, d] = V[group_j key e*128+p, d]
    vb16_in = nc.dram_tensor("vb16_in", [128, NS, 2, D], F16,
                             kind="ExternalInput")
    # biasp[p, j, t]: -2 (real pair) / -1e9 (padded pair)
    biasp_in = nc.dram_tensor("biasp_in", [128, NS, cfg.maxP], F32,
                              kind="ExternalInput")
    # blend[p, j, 2s]=local coeff, [p, j, 2s+1]=alpha
    blend_in = nc.dram_tensor("blend_in", [128, NS, 4], F32,
                              kind="ExternalInput")
    ones8_in = nc.dram_tensor("ones8_in", [128, 2, 2], F8, kind="ExternalInput")
    ones16_in = nc.dram_tensor("ones16_in", [128, 2], F16, kind="ExternalInput")
    out_t = nc.dram_tensor("out_core", [cfg.NQ, D], F16, kind="ExternalOutput")

    with tile.TileContext(nc) as tc:
        with (
            tc.tile_pool(name="const", bufs=1) as cpool,
            tc.tile_pool(name="kt8", bufs=1) as ktp,
            tc.tile_pool(name="v8", bufs=1) as vp,
            tc.tile_pool(name="qt", bufs=1) as qtp,
            tc.tile_pool(name="kbt", bufs=1) as kbtp,
            tc.tile_pool(name="vb", bufs=1) as vbp,
            tc.tile_pool(name="et", bufs=8) as etp,
            tc.tile_pool(name="ml", bufs=8) as mlp,
            tc.tile_pool(name="vec", bufs=10) as vecp,
            tc.tile_pool(name="out32", bufs=6) as o32p,
            tc.tile_pool(name="out16", bufs=3) as o16p,
            tc.tile_pool(name="poc", bufs=2, space="PSUM") as poc,
            tc.tile_pool(name="pst", bufs=3, space="PSUM") as pst,
            tc.tile_pool(name="psm", bufs=1, space="PSUM") as psm,
        ):
            # ---- constants & small tables (front of the DMA queue) ----
            ones8 = cpool.tile([128, 2, 2], F8)
            nc.scalar.dma_start(ones8[:], ones8_in[:])
            ones16 = cpool.tile([128, 2], F16)
            nc.scalar.dma_start(ones16[:], ones16_in[:])
            biasp = cpool.tile([128, NS, cfg.maxP], F32)
            nc.scalar.dma_start(biasp[:], biasp_in[:])
            blend = cpool.tile([128, NS, 4], F32)
            nc.scalar.dma_start(blend[:], blend_in[:])

            # warm the Exp table while input DMAs stream
            warm = vecp.tile([128, 2], F32, tag="v", name="warm")
            nc.scalar.activation(warm[:], ones16[:], AF.Exp, scale=1.0)

            # SHIFT bias as an AP (float biases need pre-registered consts)
            shift_t = cpool.tile([128, 1], F32)
            nc.gpsimd.memset(shift_t[:], SHIFT)

            # one persistent sums tile: slot j owns [:, j] (cross s, local s
            # chains at [:, j, s] / [:, j, 2+s]); zeroed once, all matmuls
            # accumulate with start=False so slots never clobber each other
            sums_all = psm.tile([128, NS, 4, 2], F32, name="sums_all")
            nc.vector.memset(sums_all[:], 0)

            # slots processed in DESCENDING pair count: the big slot
            # absorbs the K/V streaming phase, later slots run resident
            js = cfg.js

            kt8 = ktp.tile([128, KP, 2, 4, 2, 128], F8)
            v8 = vp.tile([128, KP, 2, D], F8)
            qt8 = qtp.tile([128, NS, 4, 2, 256], F8)
            vb8 = vbp.tile([128, NS, 2, D], F8)
            qt16 = qtp.tile([128, NS, 4, 2, 256], F16)
            kbt16 = kbtp.tile([128, NS, 4, 2, 2, 128], F16)
            vb16 = vbp.tile([128, NS, 2, D], F16)

            # All big inputs go on the sync (SP) HWDGE queue in exact
            # need-order (SWDGE descriptor generation would occupy the Pool
            # ENGINE and head-of-line-block the memsets/adds). Fine-grained
            # early K/V chunks so each pair unblocks asap; per-slot boundary
            # tensors slotted by their actual need times.
            sdma = nc.sync.dma_start
            sdma(qt8[:, js[0]], qt8_in[:, js[0]])
            sdma(kt8[:, 0:1], kt8_in[:, 0:1])
            sdma(v8[:, 0:1], v8_in[:, 0:1])
            for c0, c1 in [(1, 2), (2, 3), (3, 4), (4, 5), (5, 6)]:
                if c0 < KP:
                    c1 = min(c1, KP)
                    sdma(kt8[:, c0:c1], kt8_in[:, c0:c1])
                    sdma(v8[:, c0:c1], v8_in[:, c0:c1])
            sdma(vb8[:, js[0]], vb8_in[:, js[0]])
            if len(js) > 1:
                sdma(qt8[:, js[1]], qt8_in[:, js[1]])
            for c0, c1 in [(6, 9), (9, 12)]:
                if c0 < KP:
                    c1 = min(c1, KP)
                    sdma(kt8[:, c0:c1], kt8_in[:, c0:c1])
                    sdma(v8[:, c0:c1], v8_in[:, c0:c1])
            sdma(qt16[:, js[0]], qt16_in[:, js[0]])
            sdma(kbt16[:, js[0]], kbt16_in[:, js[0]])
            if KP > 12:
                sdma(kt8[:, 12:KP], kt8_in[:, 12:KP])
                sdma(v8[:, 12:KP], v8_in[:, 12:KP])
            sdma(vb16[:, js[0]], vb16_in[:, js[0]])
            for k, jn in enumerate(js[1:], 1):
                if k > 1:
                    sdma(qt8[:, jn], qt8_in[:, jn])
                sdma(qt16[:, jn], qt16_in[:, jn])
                sdma(kbt16[:, jn], kbt16_in[:, jn])
                sdma(vb8[:, jn], vb8_in[:, jn])
                sdma(vb16[:, jn], vb16_in[:, jn])

            # ---- software-pipelined main loop over (slot, pair) ----
            pairs = [(j, t) for j in js for t in range(cfg.P[j])]
            LOOKAHEAD = 2
            state = {}       # (j, t) -> et2
            oc = {}          # slot -> [oc0, oc1]
            bnd = {}         # slot -> (mls, et2b) from hoisted boundary work

            def emit_qk(j, t):
                st2 = pst.tile([128, 2, 256], F32, tag="st", name=f"st_{j}_{t}")
                for e in range(2):
                    for dp in range(4):
                        nc.tensor.matmul(
                            st2[:, e, :], kt8[:, t, e, dp, :, :],
                            qt8[:, j, dp, :, :],
                            start=(e == 0 and dp == 0),
                            stop=(e == 1 and dp == 3),
                            perf_mode=DR, skip_group_check=True)
                et2 = etp.tile([128, 2, 256], F8, tag="et", name=f"et_{j}_{t}")
                nc.scalar.activation(et2[:], st2[:], AF.Exp,
                                     bias=biasp[:, j, t:t + 1], scale=SCALE)
                state[(j, t)] = et2

            def emit_pv(j, t):
                et2 = state.pop((j, t))
                if t == 0:
                    oc[j] = [poc.tile([128, D], F32, tag="oc",
                                      name=f"oc{s}_{j}") for s in range(2)]
                for s in range(2):
                    lhs = et2[:, :, s * 128:(s + 1) * 128]
                    for dh in range(2):
                        nc.tensor.matmul(
                            oc[j][s][:, dh * 512:(dh + 1) * 512], lhs,
                            v8[:, t, :, dh * 512:(dh + 1) * 512],
                            start=(t == 0), stop=False,
                            perf_mode=DR, skip_group_check=True)
                    nc.tensor.matmul(sums_all[:, j, s, :], lhs, ones8[:],
                                     start=False, stop=False,
                                     perf_mode=DR, skip_group_check=True)

            def emit_bqk(j):
                """Boundary scores + masked exp tiles + local sums. Needs only
                kbt16/qt16 — hoisted into the first slot's pair stream as
                stall filler while K/V stream from HBM."""
                # boundary scores fp16 in one score buffer:
                # [:, 0, :] = b0 keys x q 0:256, [:, 1, 0:128] = b1 x q 128:256
                stb = pst.tile([128, 2, 256], F32, tag="st", name=f"stb_{j}")
                for dp in range(4):
                    for u in range(2):
                        nc.tensor.matmul(
                            stb[:, 0, :], kbt16[:, j, dp, u, 0, :],
                            qt16[:, j, dp, u, :],
                            start=(dp == 0 and u == 0), stop=False,
                            skip_group_check=True)
                for dp in range(4):
                    for u in range(2):
                        nc.tensor.matmul(
                            stb[:, 1, 0:128], kbt16[:, j, dp, u, 1, :],
                            qt16[:, j, dp, u, 128:256],
                            start=False, stop=(dp == 3 and u == 1),
                            skip_group_check=True)

                # local fp16 masked exp tiles (diag blocks; rest zero)
                mls = []
                for s in range(2):
                    ml = mlp.tile([128, 128], F16, tag="ml", name=f"ml{s}_{j}")
                    nc.gpsimd.memset(ml[:], 0)
                    # stb[:,0] holds q 0:256; stb[:,1] holds q 128:256 at 0:128
                    c0 = 0
                    nc.scalar.activation(ml[0:64, 0:64],
                                         stb[0:64, s, c0:c0 + 64],
                                         AF.Exp, scale=SCALE)
                    nc.scalar.activation(ml[64:128, 64:128],
                                         stb[64:128, s, c0 + 64:c0 + 128],
                                         AF.Exp, scale=SCALE)
                    mls.append(ml)

                # masked fp8 exp rectangles for the boundary cross pair
                et2b = etp.tile([128, 2, 256], F8, tag="et", name=f"etb_{j}")
                nc.gpsimd.memset(et2b[:], 0)
                nc.scalar.activation(et2b[0:64, 0, 64:256],
                                     stb[0:64, 0, 64:256], AF.Exp,
                                     bias=shift_t[0:64], scale=SCALE)
                nc.scalar.activation(et2b[64:128, 0, 128:256],
                                     stb[64:128, 0, 128:256], AF.Exp,
                                     bias=shift_t[64:128], scale=SCALE)
                nc.scalar.activation(et2b[0:64, 1, 192:256],
                                     stb[0:64, 1, 64:128], AF.Exp,
                                     bias=shift_t[0:64], scale=SCALE)

                # local sums (shared-bank region, start=False accumulate)
                for s in range(2):
                    nc.tensor.matmul(sums_all[:, j, 2 + s, :], mls[s][:],
                                     ones16[:], start=False, stop=True,
                                     skip_group_check=True)
                bnd[j] = (mls, et2b)

            def emit_tail(j):
                mls, et2b = bnd.pop(j)
                # fold boundary cross into oc / sums as one more DR pair
                for s in range(2):
                    lhs = et2b[:, :, s * 128:(s + 1) * 128]
                    for dh in range(2):
                        nc.tensor.matmul(
                            oc[j][s][:, dh * 512:(dh + 1) * 512], lhs,
                            vb8[:, j, :, dh * 512:(dh + 1) * 512],
                            start=False, stop=True,
                            perf_mode=DR, skip_group_check=True)
                    nc.tensor.matmul(sums_all[:, j, s, :], lhs, ones8[:],
                                     start=False, stop=(s == 1),
                                     perf_mode=DR, skip_group_check=True)

                # per-sub: cross flush, local PV (L reuses the oc buffer),
                # combine, out
                for s in range(2):
                    scm = vecp.tile([128, 1], F32, tag="v")
                    nc.vector.tensor_scalar_max(scm[:],
                                                sums_all[:, j, s, 0:1], 1e-30)
                    rc = vecp.tile([128, 1], F32, tag="v")
                    nc.vector.reciprocal(rc[:], scm[:])
                    rc2 = vecp.tile([128, 1], F32, tag="v")
                    nc.vector.tensor_mul(rc2[:], rc[:],
                                         blend[:, j, 2 * s + 1:2 * s + 2])
                    slm = vecp.tile([128, 1], F32, tag="v")
                    nc.vector.tensor_scalar_max(slm[:],
                                                sums_all[:, j, 2 + s, 0:1],
                                                1e-30)
                    rl = vecp.tile([128, 1], F32, tag="v")
                    nc.vector.reciprocal(rl[:], slm[:])
                    rl2 = vecp.tile([128, 1], F32, tag="v")
                    nc.vector.tensor_mul(rl2[:], rl[:],
                                         blend[:, j, 2 * s:2 * s + 1])

                    # scale oc halves out on DVE (frees the oc buffer for L)
                    css = []
                    for dh in range(2):
                        cs = o32p.tile([128, 512], F32, tag="o32",
                                       name=f"cs{s}{dh}_{j}")
                        nc.vector.tensor_scalar_mul(
                            cs[:], oc[j][s][:, dh * 512:(dh + 1) * 512],
                            rc2[:])
                        css.append(cs)
                    L = poc.tile([128, D], F32, tag="oc", name=f"L{s}_{j}")
                    for dh in range(2):
                        nc.tensor.matmul(
                            L[:, dh * 512:(dh + 1) * 512], mls[s][:],
                            vb16[:, j, s, dh * 512:(dh + 1) * 512],
                            start=True, stop=True)
                    fin16 = o16p.tile([128, D], F16, tag="o16",
                                      name=f"fin{s}_{j}")
                    row = (2 * j + s) * 128
                    for dh in range(2):
                        lt = o32p.tile([128, 512], F32, tag="o32",
                                       name=f"lt{s}{dh}_{j}")
                        nc.vector.tensor_scalar_mul(
                            lt[:], L[:, dh * 512:(dh + 1) * 512], rl2[:])
                        nc.gpsimd.tensor_add(
                            fin16[:, dh * 512:(dh + 1) * 512], lt[:],
                            css[dh][:])
                        nc.sync.dma_start(
                            out_t[row:row + 128, dh * 512:(dh + 1) * 512],
                            fin16[:, dh * 512:(dh + 1) * 512])

            # boundary precompute insertion points: spread across the first
            # (largest) slot's pair stream, but never after the slot's tail
            deadline = {}
            for i, (j, t) in enumerate(pairs):
                if t == cfg.P[j] - 1:
                    deadline[j] = i
            ins = {}
            for j in js:
                ins.setdefault(max(deadline[j] - 2, 0), []).append(j)

            qk_idx = 0
            for i, (j, t) in enumerate(pairs):
                for jb in ins.get(i, []):
                    emit_bqk(jb)
                while qk_idx <= i + LOOKAHEAD and qk_idx < len(pairs):
                    emit_qk(*pairs[qk_idx])
                    qk_idx += 1
                emit_pv(j, t)
                if t == cfg.P[j] - 1:
                    emit_tail(j)
    nc.compile()
    return nc


def _host_inputs(cfg: Cfg, query, key, value):
    """Build per-core input maps (fp8/fp16 pre-layouts). K/V layouts are
    shared per batch; per-core tensors are group gathers."""
    D = cfg.D
    NS = cfg.n_slot
    KP = min(cfg.maxP, cfg.npair)
    B = query.shape[0]

    kv_cache = []
    for b in range(B):
        # kt8[p, t, e, dp, u, kk] = K[(2t+e)*128+kk, dp*256+u*128+p]
        k8 = key[b, :KP * 256].astype(NPF8)          # [S', D]
        k8 = k8.reshape(KP, 2, 128, 4, 2, 128)       # [t, e, kk, dp, u, p]
        kt8 = np.ascontiguousarray(k8.transpose(5, 0, 1, 3, 4, 2))
        v8 = value[b, :KP * 256].astype(NPF8).reshape(KP, 2, 128, D)
        v8 = np.ascontiguousarray(v8.transpose(2, 0, 1, 3))  # [p, t, e, d]
        kv_cache.append((kt8, v8))

    ones8 = np.ones((128, 2, 2), NPF8)
    ones16 = np.ones((128, 2), np.float16)

    in_maps = []
    for core in range(cfg.n_cores):
        b = core // cfg.cores_per_batch
        qt_idx = core % cfg.cores_per_batch
        groups = [cls[qt_idx] for cls in cfg.classes]
        kt8, v8 = kv_cache[b]

        # [NS, 256, D] row gathers
        q_rows = np.stack([query[b, g * 256:(g + 1) * 256] for g in groups])
        k_rows = np.stack([key[b, g * 256:(g + 1) * 256] for g in groups])
        v_rows = np.stack([value[b, g * 256:(g + 1) * 256] for g in groups])

        # qt16[p, j, dp, u, q] = Q[j, q, dp*256+u*128+p]
        qt = q_rows.astype(np.float16).reshape(NS, 256, 4, 2, 128)
        qt16 = np.ascontiguousarray(qt.transpose(4, 0, 2, 3, 1))
        qt8 = qt16.astype(NPF8)
        # kbt16[p, j, dp, u, blk, kk] = K[j, blk*128+kk, dp*256+u*128+p]
        kb = k_rows.astype(np.float16).reshape(NS, 2, 128, 4, 2, 128)
        kbt16 = np.ascontiguousarray(kb.transpose(5, 0, 3, 4, 1, 2))
        # vb16[p, j, e, d]
        vb = v_rows.astype(np.float16).reshape(NS, 2, 128, D)
        vb16 = np.ascontiguousarray(vb.transpose(2, 0, 1, 3))
        vb8 = vb16.astype(NPF8)

        biasp = np.full((128, NS, cfg.maxP), NEG, np.float32)
        blendt = np.zeros((128, NS, 4), np.float32)
        for j, g in enumerate(groups):
            biasp[:, j, :g] = SHIFT
            for s in range(2):
                for half in range(2):
                    chunk = 4 * g + 2 * s + half
                    sl = slice(half * 64, half * 64 + 64)
                    blendt[sl, j, 2 * s] = 1.0 if chunk == 0 else 0.9
                    blendt[sl, j, 2 * s + 1] = 0.0 if chunk == 0 else 0.1

        in_maps.append({
            "kt8_in": kt8, "v8_in": v8,
            "qt16_in": qt16, "qt8_in": qt8,
            "kbt16_in": kbt16, "vb16_in": vb16, "vb8_in": vb8,
            "biasp_in": biasp, "blend_in": blendt,
            "ones8_in": ones8, "ones16_in": ones16,
        })
    return in_maps


def _scatter_output(cfg: Cfg, results, B):
    out = np.empty((B, cfg.S, cfg.D), np.float32)
    for core in range(cfg.n_cores):
        b = core // cfg.cores_per_batch
        qt_idx = core % cfg.cores_per_batch
        groups = [cls[qt_idx] for cls in cfg.classes]
        oc = np.asarray(results[core]["out_core"], np.float32)
        for j, g in enumerate(groups):
            out[b, g * cfg.GQ:(g + 1) * cfg.GQ] = oc[j * cfg.GQ:(j + 1) * cfg.GQ]
    return out


_nc_cache = {}


def run(cfg: Cfg, query, key, value, trace=False, trace_kwargs=None):
    ck = cfg.S
    if ck not in _nc_cache:
        _nc_cache[ck] = build_nc(cfg)
    nc = _nc_cache[ck]
    in_maps = _host_inputs(cfg, query, key, value)
    kw = {}
    if trace:
        kw = dict(trace=True, trace_cores=list(range(cfg.n_cores)),
                  **(trace_kwargs or {}))
    res = run_bass_kernel_spmd(nc, in_maps, core_ids=list(range(cfg.n_cores)),
                               **kw)
    out = _scatter_output(cfg, res.results, query.shape[0])
    return out, res


def kernel(query, key, value):
    query = np.asarray(query, np.float32)
    key = np.asarray(key, np.float32)
    value = np.asarray(value, np.float32)
    out, _ = run(FULL, query, key, value)
    return out
